# revision 12
# baseline (speedup 1.0000x reference)
"""DisentangleLossBatch Trainium2 kernel (8 NeuronCores, data-parallel).

Math: loss = sum|mean_b(G[idx_g(b), idx_h(b)]) - I| over the 8x8 top-k
Gram matrix, G = Cn @ Cn.T, idx = top-8 indices of each token's 512 pose
logits.

v2.2: fp8 DoubleRow, batched M-build, wide matmuls.

  * top-8 per token on DVE (max8 + find_index8), u16 indices.
  * one-hot M built in f16 with value 0.5 via 4x-mode tensor_scalar
    (is_equal vs per-partition iota scalar, then mult 0.5), batched
    FOUR tiles per instruction to amortize DVE instruction overhead.
    f16 0.5 is bytes (0x00, 0x38) and 0x38 is exactly fp8e4 1.0, so
    the odd-byte fp8 view of the planes is an exact 1.0-valued one-hot.
  * mid: RT = Cn8^T M via fp8e4 DoubleRow matmuls, 512 output columns
    per instruction (one full PSUM bank) to halve weight reloads.
  * evict PSUM f32 -> SBUF fp8 on Act, one [128,1024] ACTIVATE per
    e-chunk (RT values are scaled fp8-grid codebook rows; rounding
    error matches the offline fp8 sim, 0.69% off-diagonal, tol 2e-2 --
    the diagonal is excluded on host).
  * back: RT_grp^T RT_grp fp8 DoubleRow over both e-chunks, one
    [128,128] PSUM accumulation chain for the whole core.
  * PSUM: three 2-bank rt tiles rotate; one bank holds the dot
    accumulator.
"""
import sys
import numpy as np

for _p in ("/opt/trn_rl_repo",):
    if _p not in sys.path:
        sys.path.insert(0, _p)

from contextlib import ExitStack

import concourse.bass as bass
import concourse.bacc as bacc
import concourse.tile as tile
import concourse.mybir as mybir
from concourse.bass_utils import run_bass_kernel_spmd

P = 128
N_CORES = 8
B, N, D, E = 32, 1024, 512, 256
G8 = 8
BN = B * N                       # 32768 tokens
BN_PER_CORE = BN // N_CORES      # 4096
T = BN_PER_CORE // P             # 32 tiles per core
NCOL = P * G8                    # 1024 one-hot columns per tile
NGRP = P // 16                   # 8 16-token dot groups per tile
TB = 4                           # tiles per M-build batch
NPS = 3                          # rotating 2-bank PSUM rt tiles
MIDC = 512                       # mid output columns per matmul
f32 = mybir.dt.float32
f16 = mybir.dt.float16
f8 = mybir.dt.float8e4
u16 = mybir.dt.uint16
DR = mybir.MatmulPerfMode.DoubleRow


def build_nc(debug=False):
    nc = bacc.Bacc("TRN2", target_bir_lowering=False, debug=False,
                   num_devices=N_CORES)
    pose = nc.dram_tensor("pose", [BN_PER_CORE, D], f32, kind="ExternalInput")
    cb = nc.dram_tensor("codebook", [D, E], f32, kind="ExternalInput")
    iota4 = nc.dram_tensor("iota4", [P, 4], f32, kind="ExternalInput")
    dot_out = nc.dram_tensor("dot_out", [P, P], f32, kind="ExternalOutput")
    flat_dram = nc.dram_tensor("flat_scratch", [T, NCOL], u16)

    with tile.TileContext(nc) as tc, ExitStack() as ctx:
        const_pool = ctx.enter_context(tc.tile_pool(name="const", bufs=1))
        prep_pool = ctx.enter_context(tc.tile_pool(name="prep", bufs=1))
        in_pool = ctx.enter_context(tc.tile_pool(name="in", bufs=10))
        small_pool = ctx.enter_context(tc.tile_pool(name="small", bufs=12))
        m_pool = ctx.enter_context(tc.tile_pool(name="m", bufs=3))
        rt_pool = ctx.enter_context(tc.tile_pool(name="rt", bufs=6))
        rtps_pool = ctx.enter_context(tc.tile_pool(name="rtps", bufs=1,
                                                   space="PSUM"))
        dot_pool = ctx.enter_context(tc.tile_pool(name="dot", bufs=1,
                                                  space="PSUM"))

        # ---- constants: per-partition iota scalars (f32 for TSP) ----
        iota_sb = const_pool.tile([P, 4], f32)
        nc.sync.dma_start(iota_sb[:], iota4.ap())

        # ---- codebook -> normalized rows, fp8e4 (emitted AFTER the first
        # stage_a batches so the Act table loads / sqrt / reciprocal do not
        # block the top8 pipeline warmup on the in-order engine queues) ----
        cn8 = prep_pool.tile([P, 4, E], f8)

        prep_state = {}

        def prep_codebook_1():
            # Act-side: norms (runs while DVE does the first top8 batches)
            cb_sb = prep_pool.tile([P, 4, E], f32)
            cb_v = cb.ap().rearrange("(k p) e -> k p e", p=P)
            for k in range(4):
                nc.gpsimd.dma_start(cb_sb[:, k, :], cb_v[k])
            sq = prep_pool.tile([P, E], f32)
            nrm2 = prep_pool.tile([P, 4], f32)
            for k in range(4):
                nc.scalar.activation(sq[:], cb_sb[:, k, :],
                                     mybir.ActivationFunctionType.Square,
                                     accum_out=nrm2[:, k:k + 1])
            nrm = prep_pool.tile([P, 4], f32)
            nc.scalar.sqrt(nrm[:], nrm2[:])
            prep_state["cb_sb"] = cb_sb
            prep_state["nrm"] = nrm

        def prep_codebook_2():
            # DVE reciprocal + fp8 quantize (after the first maxes)
            cb_sb = prep_state["cb_sb"]
            rnorm = prep_pool.tile([P, 4], f32)
            nc.vector.reciprocal(rnorm[:], prep_state["nrm"][:])
            for k in range(4):
                nc.scalar.activation(cn8[:, k, :], cb_sb[:, k, :],
                                     mybir.ActivationFunctionType.Copy,
                                     scale=rnorm[:, k:k + 1])

        dot_ps = dot_pool.tile([P, P], f32)
        pose_v = pose.ap().rearrange("(t p) d -> t p d", p=P)
        n_dot = T * NGRP

        # rotating PSUM rt tiles, two banks each: [128, 1024] f32
        ps_tiles = [rtps_pool.tile([P, NCOL], f32, name=f"ps{i}")
                    for i in range(NPS)]
        ps_idx = [0]

        m_tiles = {}
        rt_tiles = [None] * T

        bcf_tiles = {}

        def stage_a(tb):
            # tile batch [TB*tb, TB*tb+TB): top8 + flat idx out + bcf issue
            for t in range(TB * tb, TB * tb + TB):
                pt = in_pool.tile([P, D], f32)
                nc.sync.dma_start(pt[:], pose_v[t])
                mx = small_pool.tile([P, G8], f32)
                idx16 = small_pool.tile([P, G8], u16)
                nc.vector.max(mx[:], pt[:])
                nc.vector.max_index(idx16[:], mx[:], pt[:])
                nc.gpsimd.dma_start(flat_dram.ap()[t], idx16[:])
            bcf = m_pool.tile([P, TB * NCOL], u16)
            fv = flat_dram.ap().rearrange("t n -> (t n)")
            src = fv[TB * tb * NCOL:(TB * tb + TB) * NCOL]
            nc.gpsimd.dma_start(
                bcf[:], src.unsqueeze(0).broadcast_to([P, TB * NCOL]))
            bcf_tiles[tb] = bcf

        def stage_b(tb):
            # batched one-hot compares (DVE 4x), one batch behind stage_a
            bcf = bcf_tiles.pop(tb)
            m16 = m_pool.tile([P, 4, TB * NCOL], f16)
            for k in range(4):
                nc.vector.tensor_scalar(
                    m16[:, k, :], bcf[:], iota_sb[:, k:k + 1], 0.5,
                    op0=mybir.AluOpType.is_equal,
                    op1=mybir.AluOpType.mult)
            m_tiles[tb] = m16

        def mid(t):
            # RT[e, col] = Cn^T M via fp8 DoubleRow (2 d-chunks per pass)
            tb, part = divmod(t, TB)
            m16 = m_tiles[tb]
            m8 = m16[:].bitcast(f8)            # [P, 4, 2*TB*NCOL]
            m8 = m8.rearrange("p k (c two) -> p k c two", two=2)
            col0 = part * NCOL
            rt8 = rt_pool.tile([P, 2, NCOL], f8)
            for ec in range(2):
                sl_e = slice(ec * P, (ec + 1) * P)
                ps = ps_tiles[ps_idx[0] % NPS]
                ps_idx[0] += 1
                for c in range(NCOL // MIDC):
                    sl_o = slice(c * MIDC, (c + 1) * MIDC)
                    cc = col0 + c * MIDC
                    for kk in range(2):
                        nc.tensor.matmul(
                            ps[:, sl_o],
                            lhsT=cn8[:, 2 * kk:2 * kk + 2, sl_e],
                            rhs=m8[:, 2 * kk:2 * kk + 2, cc:cc + MIDC, 1],
                            start=(kk == 0), stop=(kk == 1),
                            perf_mode=DR)
                nc.scalar.copy(rt8[:, ec, :], ps[:])
            rt_tiles[t] = rt8

        def back(t):
            # pair dots: RT_grp^T RT_grp accumulated into one [128,128],
            # DoubleRow over the two e-chunks
            rt8 = rt_tiles[t]
            for g in range(NGRP):
                sl = slice(g * P, (g + 1) * P)
                nc.tensor.matmul(dot_ps[:],
                                 lhsT=rt8[:, :, sl],
                                 rhs=rt8[:, :, sl],
                                 start=(back.di == 0),
                                 stop=(back.di == n_dot - 1),
                                 perf_mode=DR)
                back.di += 1
        back.di = 0

        # software pipeline over tile batches: A(tb) | B(tb-2) | C(tb-3);
        # within C, mid(t+1) is emitted before back(t) so the Act eviction
        # latency never stalls the in-order PE queue.
        NTB = T // TB
        prep_codebook_1()
        back_q = []

        def stage_c(tb):
            for t in range(TB * tb, TB * (tb + 1)):
                mid(t)
                back_q.append(t)
                if len(back_q) > 1:
                    back(back_q.pop(0))
            m_tiles.pop(tb)

        for tb in range(NTB + 3):
            if tb < NTB:
                stage_a(tb)
            if tb == 0:
                prep_codebook_2()
            if 2 <= tb <= NTB + 1:
                stage_b(tb - 2)
            if tb >= 3:
                stage_c(tb - 3)
        back(back_q.pop(0))

        out_sb = prep_pool.tile([P, P], f32)
        nc.scalar.copy(out_sb[:], dot_ps[:])
        nc.sync.dma_start(dot_out.ap(), out_sb[:])

    nc.compile()
    return nc


_NC_CACHE = None


def _get_nc():
    global _NC_CACHE
    if _NC_CACHE is None:
        _NC_CACHE = build_nc()
    return _NC_CACHE


def make_in_maps(pose_code: np.ndarray, codebook: np.ndarray):
    flat = np.ascontiguousarray(
        pose_code.reshape(BN, D).astype(np.float32, copy=False))
    cbf = np.ascontiguousarray(codebook.astype(np.float32, copy=False))
    iota4 = (np.arange(P)[:, None] + 128 * np.arange(4)[None, :]).astype(
        np.float32)
    in_maps = []
    for c in range(N_CORES):
        in_maps.append({
            "pose": flat[c * BN_PER_CORE:(c + 1) * BN_PER_CORE],
            "codebook": cbf,
            "iota4": iota4,
        })
    return in_maps


def finish_host(dots) -> np.ndarray:
    """Cross-core unshard: sum PE diag blocks -> loss."""
    S = np.zeros((G8, G8), dtype=np.float64)
    for d in dots:
        d4 = np.asarray(d, dtype=np.float64).reshape(16, G8, 16, G8)
        S += np.einsum("jgjh->gh", d4)
    m = S / float(BN)
    iu = np.triu_indices(G8, k=1)
    loss = 2.0 * np.abs(m[iu]).sum()
    return np.float32(loss)


def kernel(pose_code: np.ndarray, codebook: np.ndarray) -> np.ndarray:
    nc = _get_nc()
    in_maps = make_in_maps(pose_code, codebook)
    res = run_bass_kernel_spmd(nc, in_maps, core_ids=list(range(N_CORES)))
    loss = finish_host([res.results[c]["dot_out"] for c in range(N_CORES)])
    return loss.reshape(()).astype(np.float32)


# revision 13
# speedup vs baseline: 1.0121x; 1.0121x over previous
"""DisentangleLossBatch Trainium2 kernel (8 NeuronCores, data-parallel).

Math: loss = sum|mean_b(G[idx_g(b), idx_h(b)]) - I| over the 8x8 top-k
Gram matrix, G = Cn @ Cn.T, idx = top-8 indices of each token's 512 pose
logits.

v2.2: fp8 DoubleRow, batched M-build, wide matmuls.

  * top-8 per token on DVE (max8 + find_index8), u16 indices.
  * one-hot M built in f16 with value 0.5 via 4x-mode tensor_scalar
    (is_equal vs per-partition iota scalar, then mult 0.5), batched
    FOUR tiles per instruction to amortize DVE instruction overhead.
    f16 0.5 is bytes (0x00, 0x38) and 0x38 is exactly fp8e4 1.0, so
    the odd-byte fp8 view of the planes is an exact 1.0-valued one-hot.
  * mid: RT = Cn8^T M via fp8e4 DoubleRow matmuls, 512 output columns
    per instruction (one full PSUM bank) to halve weight reloads.
  * evict PSUM f32 -> SBUF fp8 on Act, one [128,1024] ACTIVATE per
    e-chunk (RT values are scaled fp8-grid codebook rows; rounding
    error matches the offline fp8 sim, 0.69% off-diagonal, tol 2e-2 --
    the diagonal is excluded on host).
  * back: RT_grp^T RT_grp fp8 DoubleRow over both e-chunks, one
    [128,128] PSUM accumulation chain for the whole core.
  * PSUM: three 2-bank rt tiles rotate; one bank holds the dot
    accumulator.
"""
import sys
import numpy as np

for _p in ("/opt/trn_rl_repo",):
    if _p not in sys.path:
        sys.path.insert(0, _p)

from contextlib import ExitStack

import concourse.bass as bass
import concourse.bacc as bacc
import concourse.tile as tile
import concourse.mybir as mybir
from concourse.bass_utils import run_bass_kernel_spmd

P = 128
N_CORES = 8
B, N, D, E = 32, 1024, 512, 256
G8 = 8
BN = B * N                       # 32768 tokens
BN_PER_CORE = BN // N_CORES      # 4096
T = BN_PER_CORE // P             # 32 tiles per core
NCOL = P * G8                    # 1024 one-hot columns per tile
NGRP = P // 16                   # 8 16-token dot groups per tile
TB = 4                           # tiles per M-build batch
NPS = 3                          # rotating 2-bank PSUM rt tiles
MIDC = 512                       # mid output columns per matmul
f32 = mybir.dt.float32
f16 = mybir.dt.float16
f8 = mybir.dt.float8e4
u16 = mybir.dt.uint16
DR = mybir.MatmulPerfMode.DoubleRow


def build_nc(debug=False):
    nc = bacc.Bacc("TRN2", target_bir_lowering=False, debug=False,
                   num_devices=N_CORES)
    pose = nc.dram_tensor("pose", [BN_PER_CORE, D], f32, kind="ExternalInput")
    cb = nc.dram_tensor("codebook", [D, E], f32, kind="ExternalInput")
    iota4 = nc.dram_tensor("iota4", [P, 4], f32, kind="ExternalInput")
    dot_out = nc.dram_tensor("dot_out", [P, P], f32, kind="ExternalOutput")
    flat_dram = nc.dram_tensor("flat_scratch", [T, NCOL], u16)

    with tile.TileContext(nc) as tc, ExitStack() as ctx:
        const_pool = ctx.enter_context(tc.tile_pool(name="const", bufs=1))
        prep_pool = ctx.enter_context(tc.tile_pool(name="prep", bufs=1))
        in_pool = ctx.enter_context(tc.tile_pool(name="in", bufs=10))
        small_pool = ctx.enter_context(tc.tile_pool(name="small", bufs=12))
        m_pool = ctx.enter_context(tc.tile_pool(name="m", bufs=3))
        rt_pool = ctx.enter_context(tc.tile_pool(name="rt", bufs=6))
        rtps_pool = ctx.enter_context(tc.tile_pool(name="rtps", bufs=1,
                                                   space="PSUM"))
        dot_pool = ctx.enter_context(tc.tile_pool(name="dot", bufs=1,
                                                  space="PSUM"))

        # ---- constants: per-partition iota scalars (f32 for TSP) ----
        iota_sb = const_pool.tile([P, 4], f32)
        nc.sync.dma_start(iota_sb[:], iota4.ap())

        # ---- codebook -> normalized rows, fp8e4 (emitted AFTER the first
        # stage_a batches so the Act table loads / sqrt / reciprocal do not
        # block the top8 pipeline warmup on the in-order engine queues) ----
        cn8 = prep_pool.tile([P, 4, E], f8)

        prep_state = {}

        def prep_codebook_1():
            # Act-side: norms (runs while DVE does the first top8 batches)
            cb_sb = prep_pool.tile([P, 4, E], f32)
            cb_v = cb.ap().rearrange("(k p) e -> k p e", p=P)
            for k in range(4):
                nc.gpsimd.dma_start(cb_sb[:, k, :], cb_v[k])
            sq = prep_pool.tile([P, E], f32)
            nrm2 = prep_pool.tile([P, 4], f32)
            for k in range(4):
                nc.scalar.activation(sq[:], cb_sb[:, k, :],
                                     mybir.ActivationFunctionType.Square,
                                     accum_out=nrm2[:, k:k + 1])
            nrm = prep_pool.tile([P, 4], f32)
            nc.scalar.sqrt(nrm[:], nrm2[:])
            prep_state["cb_sb"] = cb_sb
            prep_state["nrm"] = nrm

        def prep_codebook_2():
            # DVE reciprocal + fp8 quantize (after the first maxes)
            cb_sb = prep_state["cb_sb"]
            rnorm = prep_pool.tile([P, 4], f32)
            nc.vector.reciprocal(rnorm[:], prep_state["nrm"][:])
            for k in range(4):
                nc.scalar.activation(cn8[:, k, :], cb_sb[:, k, :],
                                     mybir.ActivationFunctionType.Copy,
                                     scale=rnorm[:, k:k + 1])

        dot_ps = dot_pool.tile([P, P], f32)
        pose_v = pose.ap().rearrange("(t p) d -> t p d", p=P)
        n_dot = T * NGRP

        # rotating PSUM rt tiles, two banks each: [128, 1024] f32
        ps_tiles = [rtps_pool.tile([P, NCOL], f32, name=f"ps{i}")
                    for i in range(NPS)]
        ps_idx = [0]

        m_tiles = {}
        rt_tiles = [None] * T

        bcf_tiles = {}

        def stage_a(tb):
            # tile batch [TB*tb, TB*tb+TB): top8 + flat idx out + bcf issue
            for t in range(TB * tb, TB * tb + TB):
                pt = in_pool.tile([P, D], f32)
                nc.sync.dma_start(pt[:], pose_v[t])
                mx = small_pool.tile([P, G8], f32)
                idx16 = small_pool.tile([P, G8], u16)
                nc.vector.max(mx[:], pt[:])
                nc.vector.max_index(idx16[:], mx[:], pt[:])
                nc.gpsimd.dma_start(flat_dram.ap()[t], idx16[:])
            bcf = m_pool.tile([P, TB * NCOL], u16)
            fv = flat_dram.ap().rearrange("t n -> (t n)")
            src = fv[TB * tb * NCOL:(TB * tb + TB) * NCOL]
            nc.gpsimd.dma_start(
                bcf[:], src.unsqueeze(0).broadcast_to([P, TB * NCOL]))
            bcf_tiles[tb] = bcf

        def stage_b(tb):
            # batched one-hot compares (DVE 4x), one batch behind stage_a
            bcf = bcf_tiles.pop(tb)
            m16 = m_pool.tile([P, 4, TB * NCOL], f16)
            for k in range(4):
                nc.vector.tensor_scalar(
                    m16[:, k, :], bcf[:], iota_sb[:, k:k + 1], 0.5,
                    op0=mybir.AluOpType.is_equal,
                    op1=mybir.AluOpType.mult)
            m_tiles[tb] = m16

        def mid(t):
            # RT[e, col] = Cn^T M via fp8 DoubleRow (2 d-chunks per pass)
            tb, part = divmod(t, TB)
            m16 = m_tiles[tb]
            m8 = m16[:].bitcast(f8)            # [P, 4, 2*TB*NCOL]
            m8 = m8.rearrange("p k (c two) -> p k c two", two=2)
            col0 = part * NCOL
            rt8 = rt_pool.tile([P, 2, NCOL], f8)
            for ec in range(2):
                sl_e = slice(ec * P, (ec + 1) * P)
                ps = ps_tiles[ps_idx[0] % NPS]
                ps_idx[0] += 1
                for c in range(NCOL // MIDC):
                    sl_o = slice(c * MIDC, (c + 1) * MIDC)
                    cc = col0 + c * MIDC
                    for kk in range(2):
                        nc.tensor.matmul(
                            ps[:, sl_o],
                            lhsT=cn8[:, 2 * kk:2 * kk + 2, sl_e],
                            rhs=m8[:, 2 * kk:2 * kk + 2, cc:cc + MIDC, 1],
                            start=(kk == 0), stop=(kk == 1),
                            perf_mode=DR)
                nc.scalar.copy(rt8[:, ec, :], ps[:])
            rt_tiles[t] = rt8

        def back(t):
            # pair dots: RT_grp^T RT_grp accumulated into one [128,128],
            # DoubleRow over the two e-chunks
            rt8 = rt_tiles[t]
            for g in range(NGRP):
                sl = slice(g * P, (g + 1) * P)
                nc.tensor.matmul(dot_ps[:],
                                 lhsT=rt8[:, :, sl],
                                 rhs=rt8[:, :, sl],
                                 start=(back.di == 0),
                                 stop=(back.di == n_dot - 1),
                                 perf_mode=DR)
                back.di += 1
        back.di = 0

        # software pipeline over tile batches: A(tb) | B(tb-2) | C(tb-3);
        # within C, mid(t+1) is emitted before back(t) so the Act eviction
        # latency never stalls the in-order PE queue.
        NTB = T // TB
        prep_codebook_1()
        back_q = []

        def stage_c(tb):
            for t in range(TB * tb, TB * (tb + 1)):
                mid(t)
                back_q.append(t)
                if len(back_q) > 1:
                    back(back_q.pop(0))
            m_tiles.pop(tb)

        for tb in range(NTB + 3):
            if tb < NTB:
                stage_a(tb)
            if tb == 1:
                prep_codebook_2()
            if 2 <= tb <= NTB + 1:
                stage_b(tb - 2)
            if tb >= 3:
                stage_c(tb - 3)
        back(back_q.pop(0))

        out_sb = prep_pool.tile([P, P], f32)
        nc.scalar.copy(out_sb[:], dot_ps[:])
        nc.sync.dma_start(dot_out.ap(), out_sb[:])

    nc.compile()
    return nc


_NC_CACHE = None


def _get_nc():
    global _NC_CACHE
    if _NC_CACHE is None:
        _NC_CACHE = build_nc()
    return _NC_CACHE


def make_in_maps(pose_code: np.ndarray, codebook: np.ndarray):
    flat = np.ascontiguousarray(
        pose_code.reshape(BN, D).astype(np.float32, copy=False))
    cbf = np.ascontiguousarray(codebook.astype(np.float32, copy=False))
    iota4 = (np.arange(P)[:, None] + 128 * np.arange(4)[None, :]).astype(
        np.float32)
    in_maps = []
    for c in range(N_CORES):
        in_maps.append({
            "pose": flat[c * BN_PER_CORE:(c + 1) * BN_PER_CORE],
            "codebook": cbf,
            "iota4": iota4,
        })
    return in_maps


def finish_host(dots) -> np.ndarray:
    """Cross-core unshard: sum PE diag blocks -> loss."""
    S = np.zeros((G8, G8), dtype=np.float64)
    for d in dots:
        d4 = np.asarray(d, dtype=np.float64).reshape(16, G8, 16, G8)
        S += np.einsum("jgjh->gh", d4)
    m = S / float(BN)
    iu = np.triu_indices(G8, k=1)
    loss = 2.0 * np.abs(m[iu]).sum()
    return np.float32(loss)


def kernel(pose_code: np.ndarray, codebook: np.ndarray) -> np.ndarray:
    nc = _get_nc()
    in_maps = make_in_maps(pose_code, codebook)
    res = run_bass_kernel_spmd(nc, in_maps, core_ids=list(range(N_CORES)))
    loss = finish_host([res.results[c]["dot_out"] for c in range(N_CORES)])
    return loss.reshape(()).astype(np.float32)
